# revision 33
# baseline (speedup 1.0000x reference)
"""Trainium2 Bass kernel for the MDA GNN (3x GAT views + MS-CAM fusion + pair MLP).

v2 design, 8 NeuronCores = 4 row-quarters (a) x 2 feature-halves (b):

  S1   h[j, :] = feat @ [8*W_half | 8*wsrc | 8*wdst] for my j-quarter only,
       fp8e4 DoubleRow matmuls (k packed in pairs of 128), h stays in SBUF.
  tiny AllGather of the adst column ([CJ,1] -> [N,1]) per view (4-rank groups)
  p    p[j_local, i] = exp(leakyrelu(adst_i + asrc_j) - ln4) * mask[j, i]
       for ALL 1778 fused targets i (scalar engine Prelu+Exp, DVE mask mul)
  S2   partial S[i, :] = sum_{j local} p[j,i] h[j,:], fp8 DoubleRow,
       16 i-tiles of 112 rows; ReduceScatter(add) over the 4-rank group
       gives each core its i-quarter summed over all sources.
  CAM  channel attention on [112x4, 452] tiles (global BN stats via 2 tiny
       AllGathers), final q,r projections; host sums the two halves and
       applies the collapsed (linear) pair MLP + gather.
"""

import numpy as np
import ml_dtypes

import concourse.bass as bass
import concourse.mybir as mybir
import concourse.tile as tile
from concourse import bacc
from concourse.bass_utils import run_bass_kernel_spmd

F8 = mybir.dt.float8e4
BF16 = mybir.dt.bfloat16
F32 = mybir.dt.float32
AF = mybir.ActivationFunctionType
MUL = mybir.AluOpType.mult
ADD = mybir.AluOpType.add
DRow = mybir.MatmulPerfMode.DoubleRow

NCORES = 8
NA = 4                 # row quarters
OUT = 901
OH = 452               # padded half width (904 = 2*452)
HC = 456               # h block cols: 0..451 h-half, 452 asrc, 453 adst, 454 ones, 455 pad
MI = 1792              # padded fused target rows (16 * 112)
IT = 112               # i-tile rows
NIT = 16
NROWS = 1778
NPAIRS = 4096
EPS = 1e-5
CNT = float(NROWS * OUT)
WS = 8.0               # host-side W scale (dodges fp8 subnormals)
LN4 = 1.3862943611198906

VIEWS = [
    dict(name="drug", N=2060, off=1183),
    dict(name="inc", N=2459, off=1582),
    dict(name="mrna", N=3929, off=3052),
]
for V in VIEWS:
    V["CJ"] = -(-V["N"] // NA)            # per-core source rows (quarter)
    V["JG"] = V["CJ"] * NA
    V["NJS"] = -(-V["CJ"] // 128)         # source-row subtiles
    V["NJSp"] = V["NJS"] + (V["NJS"] % 2)
    V["NKP"] = -(-V["N"] // 256)          # contraction pair-tiles
    V["NK"] = 2 * V["NKP"]
    V["KP"] = V["NK"] * 128

_CACHE = {}
LAST_RESULTS = None


def _rawap(src_ap, offset, dims):
    return bass.AP(tensor=src_ap.tensor, offset=src_ap.offset + offset, ap=dims)


def _bcast(src_ap, parts, cols, offset=0):
    return _rawap(src_ap, offset, [[0, parts], [1, cols]])


def build_graph():
    nc = bacc.Bacc("TRN2", target_bir_lowering=False, debug=False,
                   enable_asserts=False, num_devices=NCORES)
    ins = {}
    for V in VIEWS:
        n = V["name"]
        ins[f"featT_{n}"] = nc.dram_tensor(
            f"featT_{n}", [128, V["NK"], V["CJ"]], F8, kind="ExternalInput").ap()
        ins[f"Wx_{n}"] = nc.dram_tensor(
            f"Wx_{n}", [128, V["NK"], HC], F8, kind="ExternalInput").ap()
        ins[f"mask_{n}"] = nc.dram_tensor(
            f"mask_{n}", [128, V["NJSp"], MI], F8, kind="ExternalInput").ap()
        ins[f"b_{n}"] = nc.dram_tensor(f"b_{n}", [1, OH], BF16, kind="ExternalInput").ap()
        ins[f"ad_{n}"] = nc.dram_tensor(f"ad_{n}", [1, MI], BF16, kind="ExternalInput").ap()
        ins[f"asrc_{n}"] = nc.dram_tensor(f"asrc_{n}", [128, V["NJS"]], F32, kind="ExternalInput").ap()
    ins["md"] = nc.dram_tensor("md", [IT, 4 * OH], BF16, kind="ExternalInput").ap()
    ins["validi"] = nc.dram_tensor("validi", [IT, 4], F32, kind="ExternalInput").ap()
    ins["camw"] = nc.dram_tensor("camw", [1, 16], F32, kind="ExternalInput").ap()
    ins["wab"] = nc.dram_tensor("wab", [2, OH], BF16, kind="ExternalInput").ap()
    qr_out = nc.dram_tensor("qr", [4 * IT, 2], F32, kind="ExternalOutput").ap()
    rg_half = [[0, 1, 2, 3], [4, 5, 6, 7]]
    rg_all = [list(range(NCORES))]

    with tile.TileContext(nc) as tc:
        with (
            tc.tile_pool(name="persist", bufs=1) as per,
            tc.tile_pool(name="stream", bufs=2) as st,
            tc.tile_pool(name="dram", bufs=1, space="DRAM") as dr,
            tc.tile_pool(name="ps_s1", bufs=2, space="PSUM") as ps1,
            tc.tile_pool(name="ps_s2", bufs=1, space="PSUM") as ps2p,
            tc.tile_pool(name="ps_sm", bufs=1, space="PSUM") as pss,
        ):
            # ---- constants / small loads ----
            ones = per.tile([128, 1], F32, tag="ones")
            nc.vector.memset(ones, 1.0)
            ones1r = per.tile([1, IT], F32, tag="ones1r")
            nc.vector.memset(ones1r, 1.0)
            epst = per.tile([1, 1], F32, tag="epst")
            nc.vector.memset(epst, EPS)
            ln4b = per.tile([128, 1], F32, tag="ln4b")
            nc.vector.memset(ln4b, -LN4)
            camb = per.tile([128, 16], F32, tag="camb")
            nc.scalar.dma_start(camb, _bcast(ins["camw"], 128, 16))
            validt = per.tile([IT, 4], F32, tag="validt")
            nc.scalar.dma_start(validt, ins["validi"][:, :])
            invalidt = per.tile([IT, 4], F32, tag="invalidt")
            nc.vector.tensor_scalar(invalidt, validt, -1.0, 1.0, op0=MUL, op1=ADD)
            mdt = per.tile([IT, 4 * OH], BF16, tag="mdt")
            nc.scalar.dma_start(mdt, ins["md"][:, :])
            wabc = per.tile([IT, 2 * OH], BF16, tag="wabc")
            nc.scalar.dma_start(wabc[:, 0:OH], _bcast(ins["wab"], IT, OH, offset=0))
            nc.scalar.dma_start(wabc[:, OH:2 * OH], _bcast(ins["wab"], IT, OH, offset=OH))
            bbc = {}
            for vi, V in enumerate(VIEWS):
                t = per.tile([IT, OH], BF16, tag=f"bbc{vi}", name=f"bbc{vi}")
                nc.scalar.dma_start(t, _bcast(ins[f"b_{V['name']}"], IT, OH))
                bbc[vi] = t

            ftT, wx, mk, hv, pt, asr, adall, adstbc = {}, {}, {}, {}, {}, {}, {}, {}
            for vi, V in enumerate(VIEWS):
                hv[vi] = per.tile([128, V["NJSp"], HC], F8, tag=f"hv{vi}",
                                  name=f"hv{vi}")
                nc.gpsimd.memset(hv[vi], 1.0)  # ones col / inert pad rows
                asr[vi] = per.tile([128, V["NJS"]], F32, tag=f"asr{vi}",
                                   name=f"asr{vi}")
                nc.scalar.dma_start(asr[vi], ins[f"asrc_{V['name']}"][:, :])
            for vi in (0, 1):  # drug/inc have odd NJS -> zero the pad block
                V = VIEWS[vi]
                pt[vi] = per.tile([128, V["NJSp"], MI], F8, tag="pt",
                                  name=f"pt{vi}", bufs=2)
                nc.gpsimd.memset(pt[vi][:, V["NJS"]: V["NJSp"], :], 0.0)
            pt[2] = per.tile([128, VIEWS[2]["NJSp"], MI], F8, tag="pt",
                             name="pt2", bufs=2)

            fts = {}

            def load_view_inputs(vi):
                V = VIEWS[vi]
                n, CJ, NK = V["name"], V["CJ"], V["NK"]
                wx[vi] = per.tile([128, NK, HC], F8, tag="wx", name=f"wx{vi}", bufs=2)
                nc.sync.dma_start(wx[vi], ins[f"Wx_{n}"][:, :, :])
                fts[vi] = []
                for ci in range(-(-V["NJS"] // 2)):
                    ft = per.tile([128, NK, 256], F8, tag="ftT",
                                  name=f"ftT{vi}_{ci}", bufs=3)
                    c0, c1 = ci * 256, min(CJ, ci * 256 + 256)
                    nc.sync.dma_start(ft[:, :, : c1 - c0],
                                      ins[f"featT_{n}"][:, :, c0:c1])
                    fts[vi].append(ft)

            def load_mask(vi):
                V = VIEWS[vi]
                mk[vi] = per.tile([128, V["NJSp"], MI], F8, tag="mk",
                                  name=f"mk{vi}", bufs=2)
                nc.scalar.dma_start(mk[vi], ins[f"mask_{V['name']}"][:, :, :])

            def stage1(vi):
                V = VIEWS[vi]
                CJ, NJS, NJSp, NKP = V["CJ"], V["NJS"], V["NJSp"], V["NKP"]
                for js in range(NJS):
                    pj = min(128, CJ - js * 128)
                    ft = fts[vi][js // 2]
                    jo = (js % 2) * 128
                    hps = ps1.tile([128, HC], F32, tag="s1ps")
                    for t in range(NKP):
                        nc.tensor.matmul(
                            hps[:pj],
                            ft[:, 2 * t: 2 * t + 2, jo: jo + pj],
                            wx[vi][:, 2 * t: 2 * t + 2, :],
                            start=(t == 0), stop=(t == NKP - 1), perf_mode=DRow)
                    # h (scaled 1/8) -> fp8, with ones column
                    nc.vector.tensor_scalar(hv[vi][:pj, js, 0:454],
                                            hps[:pj, 0:454], 1.0 / WS, None,
                                            op0=MUL)

            def pcompute(vi):
                V = VIEWS[vi]
                NJS = V["NJS"]
                ab = per.tile([128, MI], BF16, tag="adstbc", name=f"adstbc{vi}", bufs=2)
                adstbc[vi] = ab
                nc.scalar.dma_start(ab, _bcast(ins[f"ad_{V['name']}"], 128, MI))
                for js in range(NJS):
                    et = st.tile([128, MI], BF16, tag="et", bufs=2)
                    nc.scalar.activation(et, ab, AF.Prelu,
                                         bias=asr[vi][:, js: js + 1],
                                         scale=1.0, alpha=0.2)
                    nc.scalar.activation(et, et, AF.Exp, bias=ln4b)
                    nc.vector.tensor_mul(pt[vi][:, js, :], et, mk[vi][:, js, :])

            def stage2(vi):
                V = VIEWS[vi]
                NJSp = V["NJSp"]
                stg = per.tile([IT, NIT * HC], BF16, tag="stg", name=f"stg{vi}")
                for ch in range(4):
                    ps2 = [ps2p.tile([IT, HC], F32, tag=f"s2ps{k}", name=f"s2ps{k}")
                           for k in range(4)]
                    for tp in range(NJSp // 2):
                        for k in range(4):
                            ti = ch * 4 + k
                            nc.tensor.matmul(
                                ps2[k][:, :],
                                pt[vi][:, 2 * tp: 2 * tp + 2, ti * IT: (ti + 1) * IT],
                                hv[vi][:, 2 * tp: 2 * tp + 2, :],
                                start=(tp == 0), stop=(tp == NJSp // 2 - 1),
                                perf_mode=DRow)
                    for k in range(4):
                        ti = ch * 4 + k
                        dst = stg[:, ti * HC: (ti + 1) * HC]
                        nc.vector.tensor_copy(dst, ps2[k][:, :])
                rsin = dr.tile([NIT, IT, HC], BF16, tag=f"rsin{vi}", name=f"rsin{vi}")
                nc.sync.dma_start(
                    _rawap(rsin.opt(), 0, [[HC, IT], [IT * HC, NIT], [1, HC]]),
                    stg[:, :])
                rsout = dr.tile([4, IT, HC], BF16, tag=f"rsout{vi}", name=f"rsout{vi}")
                nc.gpsimd.collective_compute(
                    "ReduceScatter", ADD, replica_groups=rg_half,
                    ins=[rsin.opt()], outs=[rsout.opt()])
                return rsout

            xs = {}     # (channel, chunk) -> [IT, OH] bf16;  channel 3 = mirna_disease
            for t in range(4):
                xs[(3, t)] = mdt[:, t * OH: (t + 1) * OH]
            y1 = {}

            def epilogue(vi, rsout):
                vsb = st.tile([IT, 4, HC], BF16, tag="vsb", bufs=1)
                nc.sync.dma_start(
                    vsb,
                    _rawap(rsout.opt(), 0, [[HC, IT], [IT * HC, 4], [1, HC]]))
                rsum = st.tile([IT, 4], F32, tag="rsum")
                nc.vector.tensor_copy(rsum, vsb[:, :, 454:455])
                nc.vector.tensor_add(rsum, rsum, invalidt)
                rcp = st.tile([IT, 4], F32, tag="rcp")
                nc.vector.reciprocal(rcp, rsum)
                for t in range(4):
                    tmp = st.tile([IT, OH], BF16, tag="ep_tmp", bufs=2)
                    nc.vector.tensor_scalar_mul(tmp, vsb[:, t, 0:OH],
                                                rcp[:, t: t + 1])
                    nc.vector.tensor_add(tmp, tmp, bbc[vi])
                    xv = per.tile([IT, OH], BF16, tag=f"x{vi}{t}", name=f"x{vi}{t}")
                    nc.scalar.activation(xv, tmp, AF.Relu)
                    xs[(vi, t)] = xv
                # accumulate CAM conv1: y1_br += w_br[vi] * x
                for bi, (br, coff) in enumerate((("l", 0), ("g", 4))):
                    for t in range(4):
                        if vi == 0:
                            y = per.tile([IT, OH], BF16, tag=f"y1{br}{t}",
                                         name=f"y1{br}{t}")
                            y1[(br, t)] = y
                            nc.vector.tensor_scalar_mul(
                                y, xs[(0, t)], camb[:IT, coff: coff + 1])
                        else:
                            yt = st.tile([IT, OH], BF16, tag="y1tmp", bufs=2)
                            nc.vector.tensor_scalar_mul(
                                yt, xs[(vi, t)],
                                camb[:IT, coff + vi: coff + vi + 1])
                            nc.vector.tensor_add(y1[(br, t)], y1[(br, t)], yt)
                        if vi == 2:   # fold in channel 3 (mirna_disease)
                            yt2 = st.tile([IT, OH], BF16, tag="y1tmp", bufs=2)
                            nc.vector.tensor_scalar_mul(
                                yt2, xs[(3, t)],
                                camb[:IT, coff + 3: coff + 4])
                            nc.vector.tensor_add(y1[(br, t)], y1[(br, t)], yt2)

            # ================= main schedule =================
            load_view_inputs(0)
            load_mask(0)
            load_view_inputs(1)
            pcompute(0)
            stage1(0)
            stage1(1)
            load_mask(1)
            pcompute(1)
            rs0 = stage2(0)
            load_view_inputs(2)
            stage1(2)
            load_mask(2)
            pcompute(2)
            rs1 = stage2(1)
            rs2 = stage2(2)
            epilogue(0, rs0)
            epilogue(1, rs1)
            epilogue(2, rs2)

            # ================= CAM / MS-CAM fusion =================
            def stats_round(tag):
                # global sums over valid rows: cols (S_l, S_g, Q_l, Q_g)
                stp = pss.tile([1, 4], F32, tag="small")
                for t in range(4):
                    sc = st.tile([IT, 4], F32, tag="scst", bufs=2)
                    sq = st.tile([IT, OH], F32, tag="sqscr", bufs=1)
                    for bi, br in enumerate(("l", "g")):
                        nc.vector.reduce_sum(sc[:, bi: bi + 1], y1[(br, t)],
                                             axis=mybir.AxisListType.X)
                        nc.scalar.activation(sq, y1[(br, t)], AF.Square,
                                             accum_out=sc[:, 2 + bi: 3 + bi])
                    nc.vector.tensor_scalar_mul(sc, sc, validt[:, t: t + 1])
                    nc.tensor.matmul(stp[:1], ones[:IT], sc,
                                     start=(t == 0), stop=(t == 3))
                loc = st.tile([1, 4], F32, tag=f"loc{tag}")
                nc.vector.tensor_copy(loc, stp)
                agi = dr.tile([1, 4], F32, tag=f"sti{tag}", name=f"sti{tag}")
                ago = dr.tile([NCORES, 4], F32, tag=f"sto{tag}", name=f"sto{tag}",
                              addr_space="Shared")
                nc.sync.dma_start(agi, loc)
                nc.gpsimd.collective_compute(
                    "AllGather", mybir.AluOpType.bypass, replica_groups=rg_all,
                    ins=[agi.opt()], outs=[ago.opt()])
                gsb = st.tile([NCORES, 4], F32, tag=f"gsb{tag}")
                nc.sync.dma_start(gsb, ago[:, :])
                gps = pss.tile([1, 4], F32, tag="small2")
                nc.tensor.matmul(gps[:1], ones[:NCORES], gsb, start=True, stop=True)
                mrow = per.tile([1, 4], F32, tag=f"mrow{tag}", name=f"mrow{tag}")
                nc.scalar.mul(mrow, gps, 1.0 / CNT)
                m_ = mrow[0:1, 0:2]
                msq = st.tile([1, 2], F32, tag=f"msq{tag}")
                nc.vector.tensor_mul(msq, m_, m_)
                var = per.tile([1, 2], F32, tag=f"var{tag}", name=f"var{tag}")
                nc.vector.tensor_sub(var, mrow[0:1, 2:4], msq)
                return m_, var

            m1, var1 = stats_round("r1")
            std1 = st.tile([1, 2], F32, tag="std1")
            nc.scalar.activation(std1, var1, AF.Sqrt, bias=epst[0:1, 0:1])
            rs1_ = st.tile([1, 2], F32, tag="rs1")
            nc.vector.reciprocal(rs1_, std1)
            nmrs1 = st.tile([1, 2], F32, tag="nmrs1")
            nc.vector.tensor_mul(nmrs1, m1, rs1_)
            nc.scalar.mul(nmrs1, nmrs1, -1.0)
            pk1 = st.tile([1, 4], F32, tag="pk1")
            nc.vector.tensor_copy(pk1[:, 0:2], rs1_)
            nc.vector.tensor_copy(pk1[:, 2:4], nmrs1)
            r1ps = pss.tile([IT, 4], F32, tag="small2")
            nc.tensor.matmul(r1ps[:IT], ones1r[0:1, :IT], pk1[0:1, 0:4],
                             start=True, stop=True)
            r1bc = per.tile([IT, 4], F32, tag="r1bc")
            nc.vector.tensor_copy(r1bc, r1ps)
            # y1 = relu(y1 * rs + (-m*rs))  (in place)
            for bi, br in enumerate(("l", "g")):
                for t in range(4):
                    nc.scalar.activation(y1[(br, t)], y1[(br, t)],
                                         AF.Relu, scale=r1bc[:, bi: bi + 1],
                                         bias=r1bc[:, 2 + bi: 3 + bi])

            mr, vr = stats_round("r2")
            # per-channel alpha_l, alpha_g, beta  [1,4] each
            al = {}
            for bi, (br, coff) in enumerate((("l", 8), ("g", 12))):
                w2 = camb[0:1, coff: coff + 4]
                w2sq = st.tile([1, 4], F32, tag=f"w2sq{br}", name=f"w2sq{br}")
                nc.vector.tensor_mul(w2sq, w2, w2)
                nc.vector.tensor_scalar(w2sq, w2sq, vr[0:1, bi: bi + 1], EPS,
                                        op0=MUL, op1=ADD)
                nc.scalar.activation(w2sq, w2sq, AF.Sqrt)
                nc.vector.reciprocal(w2sq, w2sq)
                a_ = st.tile([1, 4], F32, tag=f"al{br}", name=f"al{br}")
                nc.vector.tensor_mul(a_, w2, w2sq)
                al[br] = a_
            beta = st.tile([1, 4], F32, tag="beta")
            bt = st.tile([1, 4], F32, tag="bt")
            nc.vector.tensor_scalar_mul(beta, al["l"], mr[0:1, 0:1])
            nc.vector.tensor_scalar_mul(bt, al["g"], mr[0:1, 1:2])
            nc.vector.tensor_add(beta, beta, bt)
            nc.scalar.mul(beta, beta, -1.0)
            pk2 = st.tile([1, 12], F32, tag="pk2")
            nc.vector.tensor_copy(pk2[:, 0:4], al["l"])
            nc.vector.tensor_copy(pk2[:, 4:8], al["g"])
            nc.vector.tensor_copy(pk2[:, 8:12], beta)
            r2ps = pss.tile([IT, 12], F32, tag="small2")
            nc.tensor.matmul(r2ps[:IT], ones1r[0:1, :IT], pk2[0:1, 0:12],
                             start=True, stop=True)
            r2bc = per.tile([IT, 12], F32, tag="r2bc")
            nc.vector.tensor_copy(r2bc, r2ps)

            # fuse: acc = sum_c x_c * sigmoid(al_c*u + ag_c*w + beta_c), then q,r
            qrt = per.tile([IT, 4, 2], F32, tag="qrt")
            for t in range(4):
                acc = per.tile([IT, OH], BF16, tag="acc", name=f"acc{t}", bufs=2)
                z2 = st.tile([IT, OH], BF16, tag="z2", bufs=2)
                zss = []
                for c in range(4):
                    zc = st.tile([IT, OH], BF16, tag="zc", bufs=4)
                    zc2 = st.tile([IT, OH], BF16, tag="zc2", bufs=4)
                    nc.vector.tensor_scalar_mul(zc, y1[("g", t)],
                                                r2bc[:, 4 + c: 5 + c])
                    nc.vector.tensor_scalar_mul(zc2, y1[("l", t)],
                                                r2bc[:, c: c + 1])
                    nc.vector.tensor_add(zc, zc, zc2)
                    zs = st.tile([IT, OH], BF16, tag="zs", bufs=4)
                    nc.scalar.activation(zs, zc, AF.Sigmoid,
                                         bias=r2bc[:, 8 + c: 9 + c])
                    zss.append(zs)
                for c in range(4):
                    if c == 0:
                        nc.vector.tensor_mul(acc, xs[(c, t)], zss[0])
                    else:
                        nc.vector.tensor_mul(z2, xs[(c, t)], zss[c])
                        nc.vector.tensor_add(acc, acc, z2)
                nc.vector.tensor_mul(z2, acc, wabc[:, 0:OH])
                nc.vector.reduce_sum(qrt[:, t, 0:1], z2, axis=mybir.AxisListType.X)
                nc.vector.tensor_mul(z2, acc, wabc[:, OH:2 * OH])
                nc.vector.reduce_sum(qrt[:, t, 1:2], z2, axis=mybir.AxisListType.X)
            nc.sync.dma_start(
                _rawap(qr_out, 0, [[2, IT], [2 * IT, 4], [1, 2]]),
                qrt[:, :, :])
    nc.compile()
    return nc


# ======================= host side ==================================

def _pack_k(arr, nk, cols):
    # [NK*128, C] -> [128, NK, C]
    return np.ascontiguousarray(arr.reshape(nk, 128, cols).transpose(1, 0, 2))


def _prep(inputs):
    f8 = ml_dtypes.float8_e4m3
    per_core = [dict() for _ in range(NCORES)]

    # fused-row -> graph-row mapping per view handled below; per-core i rows:
    # core (a, b) owns fused rows [a*448, (a+1)*448), b-th feature half.
    fused = np.arange(MI)
    validf = fused < NROWS

    for V in VIEWS:
        n, N, off, CJ, NK, KP, NJSp = (V["name"], V["N"], V["off"], V["CJ"],
                                       V["NK"], V["KP"], V["NJSp"])
        feat = np.asarray(inputs[f"feat_{n}"], np.float32)
        adj = np.asarray(inputs[f"adj_{n}"])
        W = np.asarray(inputs[f"W_{n}"], np.float64)
        a_src = np.asarray(inputs[f"a_src_{n}"], np.float64)
        a_dst = np.asarray(inputs[f"a_dst_{n}"], np.float64)

        g = np.where(fused < OUT, fused, off + fused - OUT)
        g = np.clip(g, 0, N - 1)

        M = (adj != 0).astype(np.float32)
        np.fill_diagonal(M, 1.0)
        Mv = M[:, g] * validf[None, :].astype(np.float32)   # [N, MI]

        WTp = np.zeros((KP, 2 * OH), np.float64)
        WTp[:N, :OUT] = W.T
        wsrc = np.zeros((KP,), np.float64)
        wsrc[:N] = W.T @ a_src
        wdst = np.zeros((KP,), np.float64)
        wdst[:N] = W.T @ a_dst

        asrc_full = feat.astype(np.float64) @ wsrc[:N]
        adst_full = feat.astype(np.float64) @ wdst[:N]
        ad_row = np.zeros((1, MI), np.float64)
        ad_row[0, :NROWS] = adst_full[g[:NROWS]]
        featT_a, mask_a, asrc_a = [], [], []
        for a in range(NA):
            j0, j1 = a * CJ, min((a + 1) * CJ, N)
            ft = np.zeros((KP, CJ), np.float32)
            ft[:N, : j1 - j0] = feat[j0:j1].T
            featT_a.append(_pack_k(ft, NK, CJ).astype(f8))
            mkr = np.zeros((NJSp * 128, MI), np.float32)
            mkr[: j1 - j0] = Mv[j0:j1]
            mask_a.append(np.ascontiguousarray(
                mkr.reshape(NJSp, 128, MI).transpose(1, 0, 2)).astype(f8))
            NJS = V["NJS"]
            asv = np.zeros((NJS * 128,), np.float32)
            asv[: j1 - j0] = asrc_full[j0:j1]
            asrc_a.append(np.ascontiguousarray(
                asv.reshape(NJS, 128).T).astype(np.float32))
        wx_b = []
        for b in range(2):
            Wx = np.zeros((KP, HC), np.float64)
            Wx[:, 0:OH] = WTp[:, b * OH: (b + 1) * OH]
            Wx[:, OH] = wsrc
            Wx[:, OH + 1] = wdst
            wx_b.append(_pack_k((WS * Wx).astype(np.float32), NK, HC).astype(f8))
        bpad = np.zeros((2 * OH,), np.float32)
        bpad[:OUT] = np.asarray(inputs[f"b_{n}"], np.float32)
        for c in range(NCORES):
            a, b = c % NA, c // NA
            per_core[c][f"featT_{n}"] = featT_a[a]
            per_core[c][f"mask_{n}"] = mask_a[a]
            per_core[c][f"Wx_{n}"] = wx_b[b]
            per_core[c][f"b_{n}"] = bpad[b * OH: (b + 1) * OH].reshape(1, OH).astype(ml_dtypes.bfloat16)
            per_core[c][f"ad_{n}"] = ad_row.astype(ml_dtypes.bfloat16)
            per_core[c][f"asrc_{n}"] = asrc_a[a]

    # collapsed pair-MLP vector + constant
    mW1 = np.asarray(inputs["mW1"], np.float64)
    mW2 = np.asarray(inputs["mW2"], np.float64)
    mW3 = np.asarray(inputs["mW3"], np.float64)
    mW4 = np.asarray(inputs["mW4"], np.float64)
    w432 = mW4 @ mW3 @ mW2
    wfull = (w432 @ mW1)[0]
    cconst = (np.asarray(inputs["mb1"], np.float64) @ w432[0]
              + np.asarray(inputs["mb2"], np.float64) @ (mW4 @ mW3)[0]
              + np.asarray(inputs["mb3"], np.float64) @ mW4[0]
              + np.asarray(inputs["mb4"], np.float64)[0])
    wap = np.zeros((2 * OH,), np.float64)
    wap[:OUT] = wfull[:OUT] / 4.0
    wbp = np.zeros((2 * OH,), np.float64)
    wbp[:OUT] = wfull[OUT:] / 4.0

    camw = np.concatenate([
        np.asarray(inputs["lw1"], np.float32).ravel(),
        np.asarray(inputs["gw1"], np.float32).ravel(),
        np.asarray(inputs["lw2"], np.float32).ravel(),
        np.asarray(inputs["gw2"], np.float32).ravel()]).reshape(1, 16)

    md = np.asarray(inputs["mirna_disease"], np.float32)
    mdp = np.zeros((MI, 2 * OH), np.float32)
    mdp[:NROWS, :OUT] = md
    bf = ml_dtypes.bfloat16
    for c in range(NCORES):
        a, b = c % NA, c // NA
        blk = mdp[a * 4 * IT: (a + 1) * 4 * IT, b * OH: (b + 1) * OH]
        per_core[c]["md"] = np.ascontiguousarray(
            blk.reshape(4, IT, OH).transpose(1, 0, 2).reshape(IT, 4 * OH)).astype(bf)
        vmask = (np.arange(4 * IT) + a * 4 * IT < NROWS).astype(np.float32)
        per_core[c]["validi"] = np.ascontiguousarray(
            vmask.reshape(4, IT).T)
        per_core[c]["camw"] = camw
        per_core[c]["wab"] = np.stack(
            [wap[b * OH: (b + 1) * OH], wbp[b * OH: (b + 1) * OH]]).astype(bf)
    return per_core, float(cconst)


def kernel(**inputs):
    global LAST_RESULTS
    if "nc" not in _CACHE:
        _CACHE["nc"] = build_graph()
    nc = _CACHE["nc"]
    in_maps, cconst = _prep(inputs)
    res = run_bass_kernel_spmd(nc, in_maps, core_ids=list(range(NCORES)))
    LAST_RESULTS = res
    qr_halves = [np.concatenate([np.asarray(res.results[b * NA + a]["qr"])
                                 for a in range(NA)]) for b in range(2)]
    qr = (qr_halves[0] + qr_halves[1]).astype(np.float64)
    q, r = qr[:NROWS, 0], qr[:NROWS, 1]
    ts = np.asarray(inputs["test_sample"])
    out = (q[ts[:, 0]] + r[ts[:, 1]] + cconst).astype(np.float32)
    return out.reshape(NPAIRS, 1)
